# revision 1
# baseline (speedup 1.0000x reference)
"""RNEA inverse dynamics v2: joint-in-partition layout, PE-matmul scans.

Layout per core: partition p = j*4 + blk (j in [0,32) joint, blk in [0,4)
sample-block), free dim = 2048 samples (4 chunks of F=512). All cumulative
sums along the kinematic chain become 128x128 block-triangular matmuls on
the otherwise-idle PE (fp32r = tf32, 1 cycle/row; inputs tf32-prerounded
on host). Per-joint constants (rp, rm, m, ibz, dmp) are folded into the
matmul weights; adds/subs fold into PSUM accumulation groups.

  uP = Wi@qA, uM = Wi@qB, Wc = Wi@qd, WcS = We@qd, Ac = Wi@qdd, AcS = We@qdd
  sin/cos via magic-round + ACT Sin (args kept within [-pi, pi])
  Ux = -(Wi.rp)@(sinP*WcS), Uy = (Wi.rp)@(cosP*WcS)
  Lx = Wi@(qd*Uy) - (Wi.rp)@(sinP*AcS), Ly = (Wi.rp)@(cosP*AcS) - Wi@(qd*Ux)
  Fx = (Wri.m)@Lx + (Wri.rm)@(sinM*Ac) + (Wri.rm)@(cosM*Wc^2) - (Wri.m)@(Wc*Uy)
  Fy = (Wri.m)@Ly - (Wri.rm)@(cosM*Ac) + (Wri.rm)@(sinM*Wc^2) + (Wri.m)@(Wc*Ux)
  tau = (Wri.ibz.Wi)@qdd + (Wri.rm)@(cosM*Ly - sinM*Lx - Wc*(cosM*Ux + sinM*Uy))
        + (Wre.rp)@(cosP*Fy - sinP*Fx) + diag(dmp)@qd
"""
import numpy as np

B, N = 65536, 32
NCORES = 8
BC = B // NCORES            # 8192 samples per core
NBLK = 4                    # sample blocks -> partition p = j*NBLK + blk
FT = BC // NBLK             # 2048 free elems per partition
F = 512                     # chunk free size (one PSUM bank)
CHUNKS = FT // F            # 4
TWO_PI = float(2 * np.pi)
MAGIC = float(1.5 * 2**23)

_CACHE = {}


def _tf32(x):
    u = np.asarray(x, np.float32).view(np.uint32).astype(np.uint64)
    u = (u + 0x1000) & 0xFFFFE000
    return u.astype(np.uint32).view(np.float32)


def _build_nc(reps=1):
    import concourse.bacc as bacc
    import concourse.bass as bass
    import concourse.mybir as mybir
    from concourse.tile import TileContext

    FP = mybir.dt.float32
    FR = mybir.dt.float32r
    BF = mybir.dt.bfloat16
    Op = mybir.AluOpType
    Act = mybir.ActivationFunctionType

    nc = bacc.Bacc()

    dins = {nm: nc.dram_tensor(nm, [N, BC], FR, kind="ExternalInput")
            for nm in ["qA", "qB", "qd", "qdd"]}
    d_wfr = nc.dram_tensor("wfr", [4, 128, 128], FR, kind="ExternalInput")
    d_wbf = nc.dram_tensor("wbf", [10, 128, 128], BF, kind="ExternalInput")
    d_tau = nc.dram_tensor("tau", [N, BC], FP, kind="ExternalOutput")

    def dview(d, c):
        # chunk c as [128 partitions (p = j*4+blk), F]: flat = p*FT + c*F + s
        return bass.AP(d, c * F, [[FT, 128], [1, F]])

    with TileContext(nc) as tc:
        with tc.tile_pool(name="wts", bufs=1) as wtp, \
             tc.tile_pool(name="inp", bufs=8) as inp, \
             tc.tile_pool(name="fpw", bufs=24) as fpw, \
             tc.tile_pool(name="bfw", bufs=20) as bfw, \
             tc.tile_pool(name="psp", bufs=8, space="PSUM") as psp:

            # ---- weights (resident) ----
            wfr = {}
            for i, nm in enumerate(["Wi", "We", "Wibz", "Wdmp"]):
                t = wtp.tile([128, 128], FR, tag=f"fr{i}", name=f"fr_{nm}")
                nc.sync.dma_start(out=t[:, :],
                                    in_=bass.AP(d_wfr, i * 128 * 128,
                                                [[128, 128], [1, 128]]))
                wfr[nm] = t
            wbf = {}
            bf_names = ["Wi", "Win", "Wrpi", "Wrpin", "Wrim", "Wrimn",
                        "Wrirm", "Wrirmn", "Wrerp", "Wrerpn"]
            for i, nm in enumerate(bf_names):
                t = wtp.tile([128, 128], BF, tag=f"bf{i}", name=f"bf_{nm}")
                nc.sync.dma_start(out=t[:, :],
                                    in_=bass.AP(d_wbf, i * 128 * 128,
                                                [[128, 128], [1, 128]]))
                wbf[nm] = t

            halfpi = wtp.tile([128, 1], FP, tag="hp", name="halfpi")
            nc.vector.memset(halfpi[:, :], float(np.pi / 2))

            mm = nc.tensor.matmul

            for rep in range(reps):
                for c in range(CHUNKS):
                    sfx = f"_{rep}_{c}"

                    def itile(nm):
                        return inp.tile([128, F], FR, tag="in", name=nm + sfx)

                    def ftile(nm):
                        return fpw.tile([128, F], FP, tag="fp", name=nm + sfx)

                    def btile(nm):
                        return bfw.tile([128, F], BF, tag="bf", name=nm + sfx)

                    def ptile(nm):
                        return psp.tile([128, F], FP, tag="ps", name=nm + sfx)

                    ins = {}
                    for nm in ["qA", "qB", "qd", "qdd"]:
                        t = itile(nm)
                        nc.sync.dma_start(out=t[:, :], in_=dview(dins[nm], c))
                        ins[nm] = t

                    # ---- scans of inputs ----
                    uP = ptile("uP")
                    mm(uP[:, :], wfr["Wi"][:, :], ins["qA"][:, :])
                    uM = ptile("uM")
                    mm(uM[:, :], wfr["Wi"][:, :], ins["qB"][:, :])
                    Wc = ptile("Wc")
                    mm(Wc[:, :], wfr["Wi"][:, :], ins["qd"][:, :])
                    WcS = ptile("WcS")
                    mm(WcS[:, :], wfr["We"][:, :], ins["qd"][:, :])
                    Ac = ptile("Ac")
                    mm(Ac[:, :], wfr["Wi"][:, :], ins["qdd"][:, :])
                    AcS = ptile("AcS")
                    mm(AcS[:, :], wfr["We"][:, :], ins["qdd"][:, :])

                    # ---- trig: sin/cos of 2*pi*u, args reduced to [-pi, pi]
                    def trig(u, pref):
                        kh = ftile(pref + "kh")
                        nc.vector.tensor_scalar(kh[:, :], u[:, :], MAGIC, None,
                                                Op.add)
                        x1 = ftile(pref + "x1")
                        nc.vector.scalar_tensor_tensor(
                            x1[:, :], kh[:, :], -MAGIC, u[:, :],
                            Op.add, Op.subtract)
                        s = ftile(pref + "s")
                        nc.scalar.activation(s[:, :], x1[:, :], Act.Sin,
                                             scale=-TWO_PI)
                        k2 = ftile(pref + "k2")
                        nc.vector.tensor_scalar(k2[:, :], u[:, :], 0.25, MAGIC,
                                                Op.add, Op.add)
                        x2 = ftile(pref + "x2")
                        nc.vector.scalar_tensor_tensor(
                            x2[:, :], k2[:, :], -MAGIC, u[:, :],
                            Op.add, Op.subtract)
                        cs = ftile(pref + "c")
                        nc.scalar.activation(cs[:, :], x2[:, :], Act.Sin,
                                             scale=-TWO_PI, bias=halfpi[:, :])
                        return s, cs

                    sinP, cosP = trig(uP, "P")
                    sinM, cosM = trig(uM, "M")

                    def tt(eng, nm, a, b, out_bf=True):
                        t = btile(nm) if out_bf else ftile(nm)
                        eng.tensor_tensor(out=t[:, :], in0=a[:, :], in1=b[:, :],
                                          op=Op.mult)
                        return t

                    V, A, G = nc.vector, nc.any, nc.gpsimd

                    # products feeding U scans
                    tt1 = tt(V, "tt1", sinP, WcS)
                    tt2 = tt(V, "tt2", cosP, WcS)
                    a1 = tt(V, "a1", sinP, AcS)
                    a2 = tt(V, "a2", cosP, AcS)

                    Ux = ptile("Ux")
                    mm(Ux[:, :], wbf["Wrpin"][:, :], tt1[:, :])
                    Uy = ptile("Uy")
                    mm(Uy[:, :], wbf["Wrpi"][:, :], tt2[:, :])

                    # Wc-derived (ACT)
                    W2 = ftile("W2")
                    nc.scalar.activation(W2[:, :], Wc[:, :], Act.Square)
                    Wcb = ftile("Wcb")
                    nc.scalar.copy(Wcb[:, :], Wc[:, :])

                    d1 = tt(A, "d1", ins["qd"].bitcast(FP), Uy)
                    d2 = tt(A, "d2", ins["qd"].bitcast(FP), Ux)
                    wux = tt(V, "wux", Wcb, Ux)
                    wuy = tt(V, "wuy", Wcb, Uy)
                    cu = tt(V, "cu", cosM, Ux, out_bf=False)
                    su = tt(V, "su", sinM, Uy, out_bf=False)
                    m1 = tt(A, "m1", sinM, Ac)
                    m2 = tt(A, "m2", cosM, Ac)
                    w1 = tt(G, "w1", cosM, W2)
                    w2 = tt(G, "w2", sinM, W2)
                    c3a = tt(G, "c3a", cu, Wcb)
                    c3b = tt(G, "c3b", su, Wcb)

                    Lx = ptile("Lx")
                    mm(Lx[:, :], wbf["Wi"][:, :], d1[:, :], start=True, stop=False)
                    mm(Lx[:, :], wbf["Wrpin"][:, :], a1[:, :], start=False, stop=True)
                    Ly = ptile("Ly")
                    mm(Ly[:, :], wbf["Wrpi"][:, :], a2[:, :], start=True, stop=False)
                    mm(Ly[:, :], wbf["Win"][:, :], d2[:, :], start=False, stop=True)

                    Lxb = btile("Lxb")
                    nc.scalar.copy(Lxb[:, :], Lx[:, :])
                    Lyb = btile("Lyb")
                    nc.scalar.copy(Lyb[:, :], Ly[:, :])
                    prod4 = tt(A, "prod4", cosM, Ly)
                    prod5 = tt(A, "prod5", sinM, Lx)

                    Fx = ptile("Fx")
                    mm(Fx[:, :], wbf["Wrim"][:, :], Lxb[:, :], start=True, stop=False)
                    mm(Fx[:, :], wbf["Wrirm"][:, :], m1[:, :], start=False, stop=False)
                    mm(Fx[:, :], wbf["Wrirm"][:, :], w1[:, :], start=False, stop=False)
                    mm(Fx[:, :], wbf["Wrimn"][:, :], wuy[:, :], start=False, stop=True)
                    Fy = ptile("Fy")
                    mm(Fy[:, :], wbf["Wrim"][:, :], Lyb[:, :], start=True, stop=False)
                    mm(Fy[:, :], wbf["Wrirmn"][:, :], m2[:, :], start=False, stop=False)
                    mm(Fy[:, :], wbf["Wrirm"][:, :], w2[:, :], start=False, stop=False)
                    mm(Fy[:, :], wbf["Wrim"][:, :], wux[:, :], start=False, stop=True)

                    e1 = tt(V, "e1", cosP, Fy)
                    e1b = tt(V, "e1b", sinP, Fx)

                    tau = ptile("tau")
                    mm(tau[:, :], wfr["Wibz"][:, :], ins["qdd"][:, :],
                       start=True, stop=False)
                    mm(tau[:, :], wfr["Wdmp"][:, :], ins["qd"][:, :],
                       start=False, stop=False)
                    mm(tau[:, :], wbf["Wrirm"][:, :], prod4[:, :],
                       start=False, stop=False)
                    mm(tau[:, :], wbf["Wrirmn"][:, :], prod5[:, :],
                       start=False, stop=False)
                    mm(tau[:, :], wbf["Wrirmn"][:, :], c3a[:, :],
                       start=False, stop=False)
                    mm(tau[:, :], wbf["Wrirmn"][:, :], c3b[:, :],
                       start=False, stop=False)
                    mm(tau[:, :], wbf["Wrerp"][:, :], e1[:, :],
                       start=False, stop=False)
                    mm(tau[:, :], wbf["Wrerpn"][:, :], e1b[:, :],
                       start=False, stop=True)

                    taub = ftile("taub")
                    nc.scalar.copy(taub[:, :], tau[:, :])
                    nc.sync.dma_start(out=dview(d_tau, c), in_=taub[:, :])

    nc.finalize()
    return nc


def _host_prep(q, qd, qdd_des, trans, mass, com, inertia, damping):
    """Returns per-core input dict pieces + weight stacks."""
    px, py = trans[:, 0].astype(np.float64), trans[:, 1].astype(np.float64)
    mc = (mass[:, None] * com).astype(np.float64)
    mcx, mcy = mc[:, 0], mc[:, 1]

    def skew(v):
        x, y, z = v[..., 0], v[..., 1], v[..., 2]
        o = np.zeros_like(x)
        return np.stack([np.stack([o, -z, y], -1),
                         np.stack([z, o, -x], -1),
                         np.stack([-y, x, o], -1)], -2)
    Sk = skew(com.astype(np.float64))
    Ibar = inertia + (mass[:, None, None] * (Sk @ np.swapaxes(Sk, -1, -2))
                      ).astype(np.float32)
    ibzz = Ibar[:, 2, 2].astype(np.float64)

    rp = np.hypot(px, py)
    alpha = np.arctan2(py, px)
    rm = np.hypot(mcx, mcy)
    beta = np.arctan2(mcy, mcx)
    inv2pi = 1.0 / (2 * np.pi)

    qA = np.empty((B, N), np.float32)
    qA[:, 0] = np.float32(alpha[0] * inv2pi)
    dal = np.diff(alpha) * inv2pi
    qA[:, 1:] = (q[:, :-1] * np.float32(inv2pi)
                 + dal.astype(np.float32)[None, :])
    dbe = np.empty(N, np.float64)
    dbe[0] = beta[0]
    dbe[1:] = np.diff(beta)
    qB = q * np.float32(inv2pi) + (dbe * inv2pi).astype(np.float32)[None, :]

    # transposed [N, B] tf32 inputs
    qA_T = _tf32(np.ascontiguousarray(qA.T))
    qB_T = _tf32(np.ascontiguousarray(qB.T))
    qd_T = _tf32(np.ascontiguousarray(qd.T))
    qdd_T = _tf32(np.ascontiguousarray(qdd_des.T))

    # 32x32 triangular blocks in j-space -> 128x128 with p = j*4+blk
    jj = np.arange(N)
    Ti = (jj[:, None] <= jj[None, :]).astype(np.float64)   # cumsum: W[i,o]=[i<=o]
    Te = (jj[:, None] < jj[None, :]).astype(np.float64)
    Tri = (jj[:, None] >= jj[None, :]).astype(np.float64)  # reverse inclusive
    Tre = (jj[:, None] > jj[None, :]).astype(np.float64)   # reverse exclusive

    def expand(T32):
        # W128[p_in, p_out] = T32[j_in, j_out] * (blk_in == blk_out)
        W = np.zeros((128, 128), np.float64)
        for blk in range(NBLK):
            W[blk::NBLK, blk::NBLK] = T32
        return W

    Wibz32 = Ti @ np.diag(ibzz) @ Tri   # [k, j] = sum_{i>=max(j,k)} ibz_i
    wfr = np.stack([
        _tf32(expand(Ti)), _tf32(expand(Te)),
        _tf32(expand(Wibz32)), _tf32(expand(np.diag(damping.astype(np.float64)))),
    ]).astype(np.float32)

    bf_list = [
        expand(Ti),                        # Wi
        expand(-Ti),                       # Win
        expand(rp[:, None] * Ti),          # Wrpi  (rp indexed by j_in)
        expand(-rp[:, None] * Ti),         # Wrpin
        expand(mass[:, None] * Tri),       # Wrim
        expand(-mass[:, None] * Tri),      # Wrimn
        expand(rm[:, None] * Tri),         # Wrirm
        expand(-rm[:, None] * Tri),        # Wrirmn
        expand(rp[:, None] * Tre),         # Wrerp
        expand(-rp[:, None] * Tre),        # Wrerpn
    ]
    import ml_dtypes
    wbf = np.stack([w.astype(np.float32) for w in bf_list]).astype(ml_dtypes.bfloat16)
    return qA_T, qB_T, qd_T, qdd_T, wfr, wbf


def kernel(q, qd, qdd_des, trans, mass, com, inertia, damping):
    from concourse.bass_utils import run_bass_kernel_spmd

    q = np.asarray(q, np.float32)
    qd = np.asarray(qd, np.float32)
    qdd = np.asarray(qdd_des, np.float32)
    qA_T, qB_T, qd_T, qdd_T, wfr, wbf = _host_prep(
        q, qd, qdd, np.asarray(trans), np.asarray(mass),
        np.asarray(com), np.asarray(inertia), np.asarray(damping))

    if "nc" not in _CACHE:
        _CACHE["nc"] = _build_nc()
    nc = _CACHE["nc"]

    in_maps = []
    for cix in range(NCORES):
        sl = slice(cix * BC, (cix + 1) * BC)
        in_maps.append({
            "qA": np.ascontiguousarray(qA_T[:, sl]),
            "qB": np.ascontiguousarray(qB_T[:, sl]),
            "qd": np.ascontiguousarray(qd_T[:, sl]),
            "qdd": np.ascontiguousarray(qdd_T[:, sl]),
            "wfr": wfr,
            "wbf": wbf,
        })
    res = run_bass_kernel_spmd(nc, in_maps, list(range(NCORES)))
    return np.concatenate([r["tau"].T for r in res.results], 0)



# revision 17
# speedup vs baseline: 1.3606x; 1.3606x over previous
"""RNEA inverse dynamics v3: joint-in-partition layout, PE-matmul scans.

Layout per core: partition p = j*4 + blk (j in [0,32) joint, blk in [0,4)
sample-block), free dim = 2048 samples (4 chunks of F=512). Cumulative sums
along the chain are 128x128 block-triangular matmuls on the PE.

v3 changes vs v2:
- 3 inputs (q fp32/tf32, qd bf16, qdd bf16) instead of 4; angle offsets
  (alpha, beta) applied via per-partition scalars in the range-reduction
  tensor_scalar, and uM derived from uP on-chip (kills the uM scan).
- range reduction: f = (u + c) mod 1, sin(2*pi*f - pi) == sin(2*pi*u + 2*pi*c')
  -> one DVE tensor_scalar + one ACT Sin per trig stream (vs 2 DVE + 1 ACT).
- scan outputs Wc/WcSr/Ac/AcSr/Ux/Uy as bf16 PSUM (2 per bank) -> DVE
  products run in 2x_1p mode (391ns vs 658ns at F=512).
- damping folded into the tau evacuation via scalar_tensor_tensor.
- tau's 4 Wrirm products collapsed to 2 via A2 = Lx + Wc*Uy, B2 = Ly - Wc*Ux.
- rp folded into the WcS/AcS scan weights (output-column scaling).
- bf16 output DMA; all weights in 3 packed DMAs.
"""
import numpy as np

B, N = 65536, 32
NCORES = 8
BC = B // NCORES            # 8192 samples per core
NBLK = 4                    # sample blocks -> partition p = j*NBLK + blk
FT = BC // NBLK             # 2048 free elems per partition
F = 512                     # chunk free size
CHUNKS = FT // F            # 4
TWO_PI = float(2 * np.pi)
INV2PI = float(1.0 / (2 * np.pi))
MAGIC = float(1.5 * 2**23)

# risk flags (validated by test.py)
PSACC_BF16 = False          # Lx/Ly/Fx/Fy/tau accumulate in bf16 psum

_CACHE = {}


def _tf32(x):
    u = np.asarray(x, np.float32).view(np.uint32).astype(np.uint64)
    u = (u + 0x1000) & 0xFFFFE000
    return u.astype(np.uint32).view(np.float32)


def _build_nc():
    import concourse.bacc as bacc
    import concourse.bass as bass
    import concourse.mybir as mybir
    from concourse.tile import TileContext

    FP = mybir.dt.float32
    FR = mybir.dt.float32r
    BF = mybir.dt.bfloat16
    Op = mybir.AluOpType
    Act = mybir.ActivationFunctionType
    ACC = BF if PSACC_BF16 else FP
    ACCW = 1024 if PSACC_BF16 else 512   # pad accumulator tiles to one bank

    nc = bacc.Bacc()

    d_qf = nc.dram_tensor("qf", [N, BC], FR, kind="ExternalInput")
    d_qdb = nc.dram_tensor("qdb", [N, BC], BF, kind="ExternalInput")
    d_qddb = nc.dram_tensor("qddb", [N, BC], BF, kind="ExternalInput")
    d_wfr = nc.dram_tensor("wfr", [128, 128], FR, kind="ExternalInput")
    d_wbf = nc.dram_tensor("wbf", [128, 11 * 128], BF, kind="ExternalInput")
    d_cst = nc.dram_tensor("cst", [128, 16], FP, kind="ExternalInput")
    d_tau = nc.dram_tensor("tau", [N, BC], BF, kind="ExternalOutput")

    def full_view(d, dt_sz_elems):
        # whole [128, FT] partition-major view of a [N, BC] dram tensor
        return bass.AP(d, 0, [[FT, 128], [1, FT]])

    with TileContext(nc) as tc:
        with tc.tile_pool(name="wts", bufs=1) as wtp, \
             tc.tile_pool(name="fpw", bufs=30) as fpw, \
             tc.tile_pool(name="bfw", bufs=81) as bfw, \
             tc.tile_pool(name="psA", bufs=4, space="PSUM") as psA, \
             tc.tile_pool(name="psL", bufs=2, space="PSUM") as psL, \
             tc.tile_pool(name="psB", bufs=2, space="PSUM") as psB:

            # ---- resident weights / constants / inputs ----
            wfr = wtp.tile([128, 128], FR, tag="wfr", name="wfr")
            nc.sync.dma_start(out=wfr[:, :],
                              in_=bass.AP(d_wfr, 0, [[128, 128], [1, 128]]))
            wbf = wtp.tile([128, 11 * 128], BF, tag="wbf", name="wbf")
            nc.sync.dma_start(out=wbf[:, :],
                              in_=bass.AP(d_wbf, 0, [[1408, 128], [1, 1408]]))
            cst = wtp.tile([128, 16], FP, tag="cst", name="cst")
            nc.sync.dma_start(out=cst[:, :],
                              in_=bass.AP(d_cst, 0, [[16, 128], [1, 16]]))

            (B_I, B_IN, B_M, B_MN, B_RM, B_RMN, B_RP, B_RPN, B_ERP,
             B_IBZ, B_DMP) = [wbf[:, i * 128:(i + 1) * 128] for i in range(11)]
            W_E2P = wfr[:, :]            # exclusive-scan / 2pi (fp32r)
            DMP = cst[:, 0:1]
            A_S, A_C, B_S, B_C = (cst[:, k:k + 1] for k in range(1, 5))
            BI_AS, BI_AC, BI_BS, BI_BC = (cst[:, k:k + 1] for k in range(5, 9))

            qf = wtp.tile([128, FT], FR, tag="qf", name="qf")
            qdb = wtp.tile([128, FT], BF, tag="qdb", name="qdb")
            qddb = wtp.tile([128, FT], BF, tag="qddb", name="qddb")
            for c in range(CHUNKS):
                s = slice(c * F, (c + 1) * F)
                for tile, dram in ((qf, d_qf), (qdb, d_qdb), (qddb, d_qddb)):
                    nc.sync.dma_start(
                        out=tile[:, s],
                        in_=bass.AP(dram, c * F, [[FT, 128], [1, F]]))
            taub = wtp.tile([128, FT], BF, tag="taub", name="taub")

            mm = nc.tensor.matmul
            V, S, G = nc.vector, nc.scalar, nc.gpsimd
            qf_fp = qf.bitcast(FP)

            st = [dict() for _ in range(CHUNKS)]   # per-chunk tile refs

            def ftile(nm, c):
                return fpw.tile([128, F], FP, tag="fp", name=f"{nm}_{c}")

            def btile(nm, c):
                return bfw.tile([128, F], BF, tag="bf", name=f"{nm}_{c}")

            def ptile(pool, nm, c):
                return pool.tile([128, F], FP, tag=pool.name,
                                 name=f"{nm}_{c}")

            def emit_front(c):
                t = st[c]
                sl = slice(c * F, (c + 1) * F)
                qf_c, qdb_c, qddb_c = qf_fp[:, sl], qdb[:, sl], qddb[:, sl]
                uP = ptile(psA, "uP", c)
                mm(uP[:, :], W_E2P, qf[:, sl])
                WcSr = ptile(psA, "WcSr", c)
                mm(WcSr[:, :], B_ERP, qdb_c)
                Ac = ptile(psA, "Ac", c)
                mm(Ac[:, :], B_I, qddb_c)
                AcSr = ptile(psA, "AcSr", c)
                mm(AcSr[:, :], B_ERP, qddb_c)
                Wc = ptile(psA, "Wc", c)
                mm(Wc[:, :], B_I, qdb_c)

                uPb = ftile("uPb", c)
                S.copy(uPb[:, :], uP[:, :])
                fMp = ftile("fMp", c)
                V.tensor_tensor(out=fMp[:, :], in0=qf_c, in1=uP[:, :],
                                op=Op.add)

                def trig(nm, u, off, bias):
                    kh = ftile("kh" + nm, c)
                    V.tensor_scalar(kh[:, :], u[:, :], off, MAGIC,
                                    Op.add, Op.add)
                    x1 = ftile("x1" + nm, c)
                    V.scalar_tensor_tensor(x1[:, :], kh[:, :], -MAGIC,
                                           u[:, :], Op.add, Op.subtract)
                    s = btile(nm, c)
                    S.activation(s[:, :], x1[:, :], Act.Sin,
                                 bias=bias, scale=-TWO_PI)
                    return s

                sinP = trig("sinP", uPb, A_S, BI_AS)
                cosP = trig("cosP", uPb, A_C, BI_AC)
                sinM = trig("sinM", fMp, B_S, BI_BS)
                cosM = trig("cosM", fMp, B_C, BI_BC)
                t.update(sinP=sinP, cosP=cosP, sinM=sinM, cosM=cosM)

                WcSrb = btile("WcSrb", c)
                S.copy(WcSrb[:, :], WcSr[:, :])
                AcSrb = btile("AcSrb", c)
                S.copy(AcSrb[:, :], AcSr[:, :])
                Wcb = btile("Wcb", c)
                S.copy(Wcb[:, :], Wc[:, :])
                W2 = btile("W2", c)
                V.tensor_tensor(out=W2[:, :], in0=Wcb[:, :], in1=Wcb[:, :],
                                op=Op.mult)
                Acb = btile("Acb", c)
                S.copy(Acb[:, :], Ac[:, :])
                t.update(W2=W2, Wcb=Wcb, Acb=Acb)

                tt1 = btile("tt1", c)
                V.tensor_tensor(out=tt1[:, :], in0=sinP[:, :],
                                in1=WcSrb[:, :], op=Op.mult)
                tt2 = btile("tt2", c)
                V.tensor_tensor(out=tt2[:, :], in0=cosP[:, :],
                                in1=WcSrb[:, :], op=Op.mult)
                a1 = btile("a1", c)
                V.tensor_tensor(out=a1[:, :], in0=sinP[:, :],
                                in1=AcSrb[:, :], op=Op.mult)
                a2 = btile("a2", c)
                V.tensor_tensor(out=a2[:, :], in0=cosP[:, :],
                                in1=AcSrb[:, :], op=Op.mult)
                t.update(a1=a1, a2=a2)

                Ux = ptile(psA, "Ux", c)
                mm(Ux[:, :], B_IN, tt1[:, :])
                Uy = ptile(psA, "Uy", c)
                mm(Uy[:, :], B_I, tt2[:, :])
                Uxb = btile("Uxb", c)
                S.copy(Uxb[:, :], Ux[:, :])
                Uyb = btile("Uyb", c)
                S.copy(Uyb[:, :], Uy[:, :])
                t.update(Uxb=Uxb, Uyb=Uyb)

            def emit_mid(c):
                t = st[c]
                sl = slice(c * F, (c + 1) * F)
                qdb_c = qdb[:, sl]
                d1 = btile("d1", c)
                G.tensor_tensor(out=d1[:, :], in0=qdb_c, in1=t["Uyb"][:, :],
                                op=Op.mult)
                d2 = btile("d2", c)
                G.tensor_tensor(out=d2[:, :], in0=qdb_c, in1=t["Uxb"][:, :],
                                op=Op.mult)
                m1 = btile("m1", c)
                G.tensor_tensor(out=m1[:, :], in0=t["Acb"][:, :],
                                in1=t["sinM"][:, :], op=Op.mult)
                m2 = btile("m2", c)
                G.tensor_tensor(out=m2[:, :], in0=t["Acb"][:, :],
                                in1=t["cosM"][:, :], op=Op.mult)
                wux = btile("wux", c)
                G.tensor_tensor(out=wux[:, :], in0=t["Wcb"][:, :],
                                in1=t["Uxb"][:, :], op=Op.mult)
                wuy = btile("wuy", c)
                G.tensor_tensor(out=wuy[:, :], in0=t["Wcb"][:, :],
                                in1=t["Uyb"][:, :], op=Op.mult)
                w1 = btile("w1", c)
                G.tensor_tensor(out=w1[:, :], in0=t["W2"][:, :],
                                in1=t["cosM"][:, :], op=Op.mult)
                w2 = btile("w2", c)
                G.tensor_tensor(out=w2[:, :], in0=t["W2"][:, :],
                                in1=t["sinM"][:, :], op=Op.mult)
                t.update(m1=m1, m2=m2, wux=wux, wuy=wuy, w1=w1, w2=w2)

                Lx = ptile(psL, "Lx", c)
                mm(Lx[:, :], B_I, d1[:, :], start=True, stop=False)
                mm(Lx[:, :], B_IN, t["a1"][:, :], start=False, stop=True)
                Ly = ptile(psL, "Ly", c)
                mm(Ly[:, :], B_I, t["a2"][:, :], start=True, stop=False)
                mm(Ly[:, :], B_IN, d2[:, :], start=False, stop=True)
                Lxb = btile("Lxb", c)
                S.copy(Lxb[:, :], Lx[:, :])
                Lyb = btile("Lyb", c)
                S.copy(Lyb[:, :], Ly[:, :])
                t.update(Lxb=Lxb, Lyb=Lyb)

            def emit_back(c):
                t = st[c]
                sl = slice(c * F, (c + 1) * F)
                qdb_c, qddb_c = qdb[:, sl], qddb[:, sl]
                A2c = btile("A2c", c)
                V.tensor_tensor(out=A2c[:, :], in0=t["Lxb"][:, :],
                                in1=t["wuy"][:, :], op=Op.add)
                B2c = btile("B2c", c)
                V.tensor_tensor(out=B2c[:, :], in0=t["Lyb"][:, :],
                                in1=t["wux"][:, :], op=Op.subtract)
                t4 = btile("t4", c)
                V.tensor_tensor(out=t4[:, :], in0=B2c[:, :],
                                in1=t["cosM"][:, :], op=Op.mult)
                t5 = btile("t5", c)
                V.tensor_tensor(out=t5[:, :], in0=A2c[:, :],
                                in1=t["sinM"][:, :], op=Op.mult)

                Fx = ptile(psB, "Fx", c)
                mm(Fx[:, :], B_M, t["Lxb"][:, :], start=True, stop=False)
                mm(Fx[:, :], B_RM, t["m1"][:, :], start=False, stop=False)
                mm(Fx[:, :], B_RM, t["w1"][:, :], start=False, stop=False)
                mm(Fx[:, :], B_MN, t["wuy"][:, :], start=False, stop=True)
                Fy = ptile(psB, "Fy", c)
                mm(Fy[:, :], B_M, t["Lyb"][:, :], start=True, stop=False)
                mm(Fy[:, :], B_RMN, t["m2"][:, :], start=False, stop=False)
                mm(Fy[:, :], B_RM, t["w2"][:, :], start=False, stop=False)
                mm(Fy[:, :], B_M, t["wux"][:, :], start=False, stop=True)

                e1 = btile("e1", c)
                V.tensor_tensor(out=e1[:, :], in0=t["cosP"][:, :],
                                in1=Fy[:, :], op=Op.mult)
                e1b = btile("e1b", c)
                V.tensor_tensor(out=e1b[:, :], in0=t["sinP"][:, :],
                                in1=Fx[:, :], op=Op.mult)

                tau = ptile(psB, "tau", c)
                mm(tau[:, :], B_IBZ, qddb_c, start=True, stop=False)
                mm(tau[:, :], B_RM, t4[:, :], start=False, stop=False)
                mm(tau[:, :], B_RMN, t5[:, :], start=False, stop=False)
                mm(tau[:, :], B_RP, e1[:, :], start=False, stop=False)
                mm(tau[:, :], B_RPN, e1b[:, :], start=False, stop=True)

                V.scalar_tensor_tensor(taub[:, sl], qdb_c, DMP, tau[:, :],
                                       Op.mult, Op.add)
                nc.sync.dma_start(
                    out=bass.AP(d_tau, c * F, [[FT, 128], [1, F]]),
                    in_=taub[:, sl])

            # software-pipelined emission: mid/back of older chunks first so
            # each engine stream interleaves three chunks
            for w in range(CHUNKS + 2):
                if w < CHUNKS:
                    emit_front(w)
                if 1 <= w <= CHUNKS:
                    emit_mid(w - 1)
                if 2 <= w:
                    emit_back(w - 2)

    nc.finalize()
    return nc


def _host_prep(q, qd, qdd_des, trans, mass, com, inertia, damping):
    import ml_dtypes
    px, py = trans[:, 0].astype(np.float64), trans[:, 1].astype(np.float64)
    mc = (mass[:, None] * com).astype(np.float64)
    mcx, mcy = mc[:, 0], mc[:, 1]

    def skew(v):
        x, y, z = v[..., 0], v[..., 1], v[..., 2]
        o = np.zeros_like(x)
        return np.stack([np.stack([o, -z, y], -1),
                         np.stack([z, o, -x], -1),
                         np.stack([-y, x, o], -1)], -2)
    Sk = skew(com.astype(np.float64))
    Ibar = inertia + (mass[:, None, None] * (Sk @ np.swapaxes(Sk, -1, -2))
                      ).astype(np.float32)
    ibzz = Ibar[:, 2, 2].astype(np.float64)

    rp = np.hypot(px, py)
    alpha = np.arctan2(py, px)
    rm = np.hypot(mcx, mcy)
    beta = np.arctan2(mcy, mcx)

    # transposed [N, B] inputs
    qf = _tf32(np.ascontiguousarray(q.T) * np.float32(INV2PI))
    qdb = np.ascontiguousarray(qd.T).astype(ml_dtypes.bfloat16)
    qddb = np.ascontiguousarray(qdd_des.T).astype(ml_dtypes.bfloat16)

    jj = np.arange(N)
    Ti = (jj[:, None] <= jj[None, :]).astype(np.float64)   # inclusive cumsum
    Te = (jj[:, None] < jj[None, :]).astype(np.float64)    # exclusive
    Tri = (jj[:, None] >= jj[None, :]).astype(np.float64)  # reverse inclusive
    Tre = (jj[:, None] > jj[None, :]).astype(np.float64)   # reverse exclusive

    def expand(T32):
        W = np.zeros((128, 128), np.float64)
        for blk in range(NBLK):
            W[blk::NBLK, blk::NBLK] = T32
        return W

    wfr = _tf32(expand(Te)).astype(np.float32)  # q is pre-scaled by 1/2pi

    Wibz32 = Ti @ np.diag(ibzz) @ Tri
    bf_list = [
        expand(Ti),                        # B_I
        expand(-Ti),                       # B_IN
        expand(mass[:, None] * Tri),       # B_M
        expand(-mass[:, None] * Tri),      # B_MN
        expand(rm[:, None] * Tri),         # B_RM
        expand(-rm[:, None] * Tri),       # B_RMN
        expand(rp[:, None] * Tre),         # B_RP
        expand(-rp[:, None] * Tre),       # B_RPN
        expand(Te * rp[None, :]),          # B_ERP (rp on out columns)
        expand(Wibz32),                    # B_IBZ
        expand(np.diag(damping.astype(np.float64))),  # B_DMP
    ]
    wbf = np.concatenate([w.astype(np.float32) for w in bf_list],
                         axis=1).astype(ml_dtypes.bfloat16)

    # per-partition constants [128, 16]
    cst = np.zeros((128, 16), np.float32)
    a2 = alpha * INV2PI
    b2 = beta * INV2PI
    offs = [a2, a2 + 0.25, b2, b2 + 0.25]
    for j in range(N):
        for blk in range(NBLK):
            p = j * NBLK + blk
            cst[p, 0] = damping[j]
            for k in range(4):
                cst[p, 1 + k] = offs[k][j]
                cst[p, 5 + k] = TWO_PI * offs[k][j]
    return qf, qdb, qddb, wfr, wbf, cst


def kernel(q, qd, qdd_des, trans, mass, com, inertia, damping):
    from concourse.bass_utils import run_bass_kernel_spmd

    q = np.asarray(q, np.float32)
    qd = np.asarray(qd, np.float32)
    qdd = np.asarray(qdd_des, np.float32)
    qf, qdb, qddb, wfr, wbf, cst = _host_prep(
        q, qd, qdd, np.asarray(trans), np.asarray(mass),
        np.asarray(com), np.asarray(inertia), np.asarray(damping))

    if "nc" not in _CACHE:
        _CACHE["nc"] = _build_nc()
    nc = _CACHE["nc"]

    in_maps = []
    for cix in range(NCORES):
        sl = slice(cix * BC, (cix + 1) * BC)
        in_maps.append({
            "qf": np.ascontiguousarray(qf[:, sl]),
            "qdb": np.ascontiguousarray(qdb[:, sl]),
            "qddb": np.ascontiguousarray(qddb[:, sl]),
            "wfr": wfr,
            "wbf": wbf,
            "cst": cst,
        })
    res = run_bass_kernel_spmd(nc, in_maps, list(range(NCORES)))
    return np.concatenate(
        [np.asarray(r["tau"], np.float32).T for r in res.results], 0)
